# revision 8
# baseline (speedup 1.0000x reference)
"""Cosine-similarity (pairwise, normalized by sqrt(|a||b|)+eps) Trainium2 kernel.

Problem: first_vector [8192, 512] f32, second_vector [8192, 512] f32,
output sim [8192, 8192] f32 with
    sim = (A @ B.T) / (sqrt(|A_n| * |B_m|) + 1e-6)        (normalize=1)

Strategy (8 NeuronCores, SPMD, no collectives):
  * 2D shard: 4-way over A rows x 2-way over B rows. Core c=(ni,mj)
    computes the [2048, 4096] output slab at (ni*2048, mj*4096).
  * On-device: per-row sum-of-squares -> scale = ssq^(-1/4); pre-scale both
    A and B rows by their scale so the GEMM output is already normalized
    (the +eps in the reference denominator is dropped; rel. impact ~5e-8).
  * Transpose scaled A/B tiles to d-major layout with PE matmuls against an
    identity (fp32 has no DMA-transpose path), then a dense fp32 GEMM:
    out[n, m] = sum_d aT[d, n] * bT[d, m], accumulated over 4 k-chunks of
    128 in PSUM, copied to SBUF and streamed out with 2 MB contiguous DMAs.
"""

import numpy as np

_N, _M, _D = 8192, 8192, 512
_P = 128
_GRID_N, _GRID_M = 4, 2
_AN = _N // _GRID_N        # A rows per core (2048)
_BM = _M // _GRID_M        # B rows per core (4096)
_KC = _D // _P             # contraction chunks (4)
_NS = 512                  # moving free dim per matmul (one PSUM bank of f32)

TRACE = False              # test harness sets True to collect an NTFF profile
LAST_RESULTS = None        # BassKernelResults of the last run (for test.py)

_NC_CACHE = {}


def _build_nc(normalize: bool):
    import concourse.bass as bass
    import concourse.mybir as mybir
    import concourse.tile as tile
    from concourse import bacc
    from concourse.masks import make_identity

    f32 = mybir.dt.float32
    # Bacc (not plain Bass): its compile() runs the TRN2 legalization passes
    # (sync-wait splitting via event semaphores, matmul->ldweights wait moves,
    # reg alloc) that walrus codegen requires.
    nc = bacc.Bacc("TRN2", target_bir_lowering=False, debug=False,
                   enable_asserts=False)

    a_d = nc.declare_dram_parameter("a", [_AN, _D], f32, isOutput=False)
    b_d = nc.declare_dram_parameter("b", [_BM, _D], f32, isOutput=False)
    out_d = nc.declare_dram_parameter("out", [_AN, _BM], f32, isOutput=True)

    KA = _AN // _P             # 16 A tiles
    KB = _BM // _P             # 32 B tiles
    SQ = mybir.ActivationFunctionType.Square

    with tile.TileContext(nc) as tc:
        with (
            tc.tile_pool(name="const", bufs=1) as const_pool,
            tc.tile_pool(name="persist", bufs=1) as persist,
            tc.tile_pool(name="stage", bufs=3) as stage,
            tc.tile_pool(name="scal", bufs=4) as scal,
            tc.tile_pool(name="wpsum", bufs=1, space=bass.MemorySpace.PSUM) as wpsum,
            tc.tile_pool(name="tpsum", bufs=3, space=bass.MemorySpace.PSUM) as tpsum,
            tc.tile_pool(name="mpsum", bufs=4, space=bass.MemorySpace.PSUM) as mpsum,
            tc.tile_pool(name="ostage", bufs=2) as ostage,
        ):
            ident = const_pool.tile([_P, _P], f32)
            make_identity(nc, ident[:])

            # d-major (transposed) scaled operands, persistent in SBUF.
            aT = persist.tile([_P, _KC, _AN], f32)     # 4 MB
            bT = persist.tile([_P, _KC, _BM], f32)     # 8 MB

            # fp32 matmuls lower to multi-uop LDWEIGHTS with a single sync-wait
            # slot, so every PE instruction may depend on at most ONE other
            # engine. This warmup matmul absorbs the identity (GpSimd) dep so
            # later transposes only ever wait on DVE.
            warm = wpsum.tile([_P, _P], f32)
            nc.tensor.matmul(warm[:], lhsT=ident[:], rhs=ident[:],
                             start=True, stop=True)

            def prep(src, n_tiles, dstT):
                """Load nat tiles, scale rows by ssq^-1/4, transpose into dstT."""
                src_r = src[:].rearrange("(g j p) d -> g p j d", p=_P, j=4)
                for g in range(n_tiles // 4):
                    nat = stage.tile([_P, 4, _D], f32, tag="nat")
                    nc.sync.dma_start(nat[:], src_r[g])
                    for j in range(4):
                        t_idx = g * 4 + j
                        if normalize:
                            # Wait-budget discipline (walrus: fp32 Matmult gets
                            # 1 sync-wait, DVE/ACT ops get 2): everything PE
                            # reads is DVE-produced; the DVE scale-mul only
                            # crosses from ACT (sqrt chain).
                            sq = stage.tile([_P, _D], f32, tag="sq")
                            ssq = scal.tile([_P, 1], f32, tag="ssq")
                            nc.vector.tensor_mul(sq[:], nat[:, j], nat[:, j])
                            nc.vector.reduce_sum(
                                ssq[:], sq[:], axis=mybir.AxisListType.X
                            )
                            rec = scal.tile([_P, 1], f32, tag="rec")
                            nc.vector.reciprocal(rec[:], ssq[:])
                            s_half = scal.tile([_P, 1], f32, tag="s_half")
                            nc.scalar.sqrt(s_half[:], rec[:])
                            s_quart = scal.tile([_P, 1], f32, tag="s_quart")
                            nc.scalar.sqrt(s_quart[:], s_half[:])
                            scaled = stage.tile([_P, _D], f32, tag="scaled")
                            nc.vector.tensor_scalar_mul(
                                scaled[:], in0=nat[:, j], scalar1=s_quart[:]
                            )
                            src_t = scaled[:]
                        else:
                            src_t = nat[:, j]
                        # Transpose [128 rows, 512 d] -> 4x [128 d, 128 rows]
                        # into one PSUM bank via matmuls against identity.
                        pt = tpsum.tile([_P, _KC, _P], f32)
                        for k in range(_KC):
                            nc.tensor.matmul(
                                pt[:, k],
                                lhsT=src_t[:, k * _P:(k + 1) * _P],
                                rhs=ident[:],
                                start=True,
                                stop=True,
                            )
                        nc.vector.tensor_copy(
                            dstT[:, :, t_idx * _P:(t_idx + 1) * _P], pt[:]
                        )

            prep(a_d, KA, aT)
            prep(b_d, KB, bT)

            # Main GEMM: out[t*128:(t+1)*128, :] = aT[:, :, t-slice].T @ bT
            for t in range(KA):
                ost = ostage.tile([_P, _BM], f32)
                for s in range(_BM // _NS):
                    ps = mpsum.tile([_P, _NS], f32)
                    for k in range(_KC):
                        nc.tensor.matmul(
                            ps[:],
                            lhsT=aT[:, k, t * _P:(t + 1) * _P],
                            rhs=bT[:, k, s * _NS:(s + 1) * _NS],
                            start=(k == 0),
                            stop=(k == _KC - 1),
                        )
                    nc.vector.tensor_copy(ost[:, s * _NS:(s + 1) * _NS], ps[:])
                nc.sync.dma_start(out_d[t * _P:(t + 1) * _P, :], ost[:])

    nc.compile()
    return nc


def _get_nc(normalize: bool):
    key = bool(normalize)
    if key not in _NC_CACHE:
        _NC_CACHE[key] = _build_nc(key)
    return _NC_CACHE[key]


def kernel(first_vector, second_vector, normalize):
    global LAST_RESULTS
    from concourse.bass_utils import run_bass_kernel_spmd

    a = np.ascontiguousarray(np.asarray(first_vector, dtype=np.float32))
    b = np.ascontiguousarray(np.asarray(second_vector, dtype=np.float32))
    assert a.shape == (_N, _D) and b.shape == (_M, _D)
    norm = bool(int(np.asarray(normalize)))

    nc = _get_nc(norm)

    in_maps = []
    for c in range(_GRID_N * _GRID_M):
        ni, mj = divmod(c, _GRID_M)
        in_maps.append(
            {
                "a": a[ni * _AN:(ni + 1) * _AN],
                "b": b[mj * _BM:(mj + 1) * _BM],
            }
        )

    res = run_bass_kernel_spmd(
        nc, in_maps, core_ids=list(range(_GRID_N * _GRID_M)), trace=TRACE
    )
    LAST_RESULTS = res

    out = np.empty((_N, _M), dtype=np.float32)
    for c in range(_GRID_N * _GRID_M):
        ni, mj = divmod(c, _GRID_M)
        out[ni * _AN:(ni + 1) * _AN, mj * _BM:(mj + 1) * _BM] = res.results[c]["out"]
    return out


# revision 9
# speedup vs baseline: 2.8640x; 2.8640x over previous
"""Cosine-similarity (pairwise, normalized by sqrt(|a||b|)+eps) Trainium2 kernel.

Problem: first_vector [8192, 512] f32, second_vector [8192, 512] f32,
output sim [8192, 8192] f32 with
    sim = (A @ B.T) / (sqrt(|A_n| * |B_m|) + 1e-6)        (normalize=1)

Strategy (8 NeuronCores, SPMD, no collectives):
  * 2D shard: 4-way over A rows x 2-way over B rows. Core c=(ni,mj)
    computes the [2048, 4096] output slab at (ni*2048, mj*4096).
  * On-device: per-row sum-of-squares -> scale = ssq^(-1/4); pre-scale both
    A and B rows by their scale so the GEMM output is already normalized
    (the +eps in the reference denominator is dropped; rel. impact ~5e-8).
  * The scale multiply also casts to fp16: fp32 matmuls on TRN2 run at 1/4
    rate (2 weight passes x half-rate 4-byte rhs stream, ~860 ns per
    128x128x512 MM measured) while fp16 runs full rate with FWL weight
    loads. fp16 operand quantization costs ~4e-4 norm-relative error.
  * Transpose scaled fp16 tiles to d-major layout with PE matmuls against an
    fp16 identity (fp32 accumulate in PSUM is exact for fp16 values), then a
    dense fp16 GEMM: out[n, m] = sum_d aT[d, n] * bT[d, m] accumulated over
    4 k-chunks of 128 in PSUM (f32), copied to SBUF (split across DVE and
    ACT) and streamed out with 2 MB contiguous DMAs.
"""

import numpy as np

_N, _M, _D = 8192, 8192, 512
_P = 128
_GRID_N, _GRID_M = 4, 2
_AN = _N // _GRID_N        # A rows per core (2048)
_BM = _M // _GRID_M        # B rows per core (4096)
_KC = _D // _P             # contraction chunks (4)
_NS = 512                  # moving free dim per matmul (one PSUM bank of f32)

TRACE = False              # test harness sets True to collect an NTFF profile
LAST_RESULTS = None        # BassKernelResults of the last run (for test.py)

_NC_CACHE = {}


def _build_nc(normalize: bool):
    import concourse.bass as bass
    import concourse.mybir as mybir
    import concourse.tile as tile
    from concourse import bacc
    from concourse.masks import make_identity

    f32 = mybir.dt.float32
    f16 = mybir.dt.float16
    # Bacc (not plain Bass): its compile() runs the TRN2 legalization passes
    # (sync-wait splitting via event semaphores, matmul->ldweights wait moves,
    # reg alloc) that walrus codegen requires.
    nc = bacc.Bacc("TRN2", target_bir_lowering=False, debug=False,
                   enable_asserts=False)

    a_d = nc.declare_dram_parameter("a", [_AN, _D], f32, isOutput=False)
    b_d = nc.declare_dram_parameter("b", [_BM, _D], f32, isOutput=False)
    out_d = nc.declare_dram_parameter("out", [_AN, _BM], f32, isOutput=True)

    KA = _AN // _P             # 16 A tiles
    KB = _BM // _P             # 32 B tiles
    SQ = mybir.ActivationFunctionType.Square

    with tile.TileContext(nc) as tc:
        with (
            tc.tile_pool(name="const", bufs=1) as const_pool,
            tc.tile_pool(name="persist", bufs=1) as persist,
            tc.tile_pool(name="stage", bufs=3) as stage,
            tc.tile_pool(name="scal", bufs=4) as scal,
            tc.tile_pool(name="wpsum", bufs=1, space=bass.MemorySpace.PSUM) as wpsum,
            tc.tile_pool(name="tpsum", bufs=3, space=bass.MemorySpace.PSUM) as tpsum,
            tc.tile_pool(name="mpsum", bufs=4, space=bass.MemorySpace.PSUM) as mpsum,
            tc.tile_pool(name="ostage", bufs=2) as ostage,
        ):
            ident = const_pool.tile([_P, _P], f16)
            make_identity(nc, ident[:])

            # d-major (transposed) scaled fp16 operands, persistent in SBUF.
            aT = persist.tile([_P, _KC, _AN], f16)     # 2 MB
            bT = persist.tile([_P, _KC, _BM], f16)     # 4 MB

            # Absorb the identity (GpSimd) dep before the transpose stream.
            warm = wpsum.tile([_P, _P], f32)
            nc.tensor.matmul(warm[:], lhsT=ident[:], rhs=ident[:],
                             start=True, stop=True)

            def prep(src, n_tiles, dstT):
                """Load nat tiles, scale rows by ssq^-1/4 (casting to fp16),
                transpose into dstT."""
                src_r = src[:].rearrange("(g j p) d -> g p j d", p=_P, j=4)
                for g in range(n_tiles // 4):
                    nat = stage.tile([_P, 4, _D], f32, tag="nat")
                    nc.sync.dma_start(nat[:], src_r[g])
                    for j in range(4):
                        t_idx = g * 4 + j
                        scaled = stage.tile([_P, _D], f16, tag="scaled")
                        if normalize:
                            sq = stage.tile([_P, _D], f32, tag="sq")
                            ssq = scal.tile([_P, 1], f32, tag="ssq")
                            nc.scalar.activation(sq[:], nat[:, j], SQ,
                                                 accum_out=ssq[:])
                            rec = scal.tile([_P, 1], f32, tag="rec")
                            nc.vector.reciprocal(rec[:], ssq[:])
                            s_half = scal.tile([_P, 1], f32, tag="s_half")
                            nc.scalar.sqrt(s_half[:], rec[:])
                            s_quart = scal.tile([_P, 1], f32, tag="s_quart")
                            nc.scalar.sqrt(s_quart[:], s_half[:])
                            nc.vector.tensor_scalar_mul(
                                scaled[:], in0=nat[:, j], scalar1=s_quart[:]
                            )
                        else:
                            nc.vector.tensor_copy(scaled[:], nat[:, j])
                        # Transpose [128 rows, 512 d] -> 4x [128 d, 128 rows]
                        # into one PSUM bank via matmuls against identity.
                        pt = tpsum.tile([_P, _KC, _P], f32)
                        for k in range(_KC):
                            nc.tensor.matmul(
                                pt[:, k],
                                lhsT=scaled[:, k * _P:(k + 1) * _P],
                                rhs=ident[:],
                                start=True,
                                stop=True,
                            )
                        nc.vector.tensor_copy(
                            dstT[:, :, t_idx * _P:(t_idx + 1) * _P], pt[:]
                        )

            prep(a_d, KA, aT)
            prep(b_d, KB, bT)

            # Main GEMM: out[t*128:(t+1)*128, :] = aT[:, :, t-slice].T @ bT
            for t in range(KA):
                ost = ostage.tile([_P, _BM], f32)
                for s in range(_BM // _NS):
                    ps = mpsum.tile([_P, _NS], f32)
                    for k in range(_KC):
                        nc.tensor.matmul(
                            ps[:],
                            lhsT=aT[:, k, t * _P:(t + 1) * _P],
                            rhs=bT[:, k, s * _NS:(s + 1) * _NS],
                            start=(k == 0),
                            stop=(k == _KC - 1),
                        )
                    # Split PSUM evacuation across DVE and ACT.
                    dst = ost[:, s * _NS:(s + 1) * _NS]
                    if (t * (_BM // _NS) + s) % 3 == 2:
                        nc.scalar.copy(dst, ps[:])
                    else:
                        nc.vector.tensor_copy(dst, ps[:])
                nc.sync.dma_start(out_d[t * _P:(t + 1) * _P, :], ost[:])

    nc.compile()
    return nc


def _get_nc(normalize: bool):
    key = bool(normalize)
    if key not in _NC_CACHE:
        _NC_CACHE[key] = _build_nc(key)
    return _NC_CACHE[key]


def kernel(first_vector, second_vector, normalize):
    global LAST_RESULTS
    from concourse.bass_utils import run_bass_kernel_spmd

    a = np.ascontiguousarray(np.asarray(first_vector, dtype=np.float32))
    b = np.ascontiguousarray(np.asarray(second_vector, dtype=np.float32))
    assert a.shape == (_N, _D) and b.shape == (_M, _D)
    norm = bool(int(np.asarray(normalize)))

    nc = _get_nc(norm)

    in_maps = []
    for c in range(_GRID_N * _GRID_M):
        ni, mj = divmod(c, _GRID_M)
        in_maps.append(
            {
                "a": a[ni * _AN:(ni + 1) * _AN],
                "b": b[mj * _BM:(mj + 1) * _BM],
            }
        )

    res = run_bass_kernel_spmd(
        nc, in_maps, core_ids=list(range(_GRID_N * _GRID_M)), trace=TRACE
    )
    LAST_RESULTS = res

    out = np.empty((_N, _M), dtype=np.float32)
    for c in range(_GRID_N * _GRID_M):
        ni, mj = divmod(c, _GRID_M)
        out[ni * _AN:(ni + 1) * _AN, mj * _BM:(mj + 1) * _BM] = res.results[c]["out"]
    return out


# revision 11
# speedup vs baseline: 2.8681x; 1.0014x over previous
"""Cosine-similarity (pairwise, normalized by sqrt(|a||b|)+eps) Trainium2 kernel.

Problem: first_vector [8192, 512] f32, second_vector [8192, 512] f32,
output sim [8192, 8192] f32 with
    sim = (A @ B.T) / (sqrt(|A_n| * |B_m|) + 1e-6)        (normalize=1)

Strategy (8 NeuronCores, SPMD, no collectives):
  * 2D shard: 4-way over A rows x 2-way over B rows. Core c=(ni,mj)
    computes the [2048, 4096] output slab at (ni*2048, mj*4096).
  * On-device: per-row sum-of-squares -> scale = ssq^(-1/4); pre-scale both
    A and B rows by their scale so the GEMM output is already normalized
    (the +eps in the reference denominator is dropped; rel. impact ~5e-8).
  * The scale multiply also casts to fp16: fp32 matmuls on TRN2 run at 1/4
    rate (2 weight passes x half-rate 4-byte rhs stream, ~860 ns per
    128x128x512 MM measured) while fp16 runs full rate with FWL weight
    loads. fp16 operand quantization costs ~4e-4 norm-relative error.
  * Transpose scaled fp16 tiles to d-major layout with PE matmuls against an
    fp16 identity (fp32 accumulate in PSUM is exact for fp16 values), then a
    dense fp16 GEMM: out[n, m] = sum_d aT[d, n] * bT[d, m] accumulated over
    4 k-chunks of 128 in PSUM (f32), copied to SBUF (split across DVE and
    ACT) and streamed out with 2 MB contiguous DMAs.
"""

import numpy as np

_N, _M, _D = 8192, 8192, 512
_P = 128
_GRID_N, _GRID_M = 4, 2
_AN = _N // _GRID_N        # A rows per core (2048)
_BM = _M // _GRID_M        # B rows per core (4096)
_KC = _D // _P             # contraction chunks (4)
_NS = 512                  # moving free dim per matmul (one PSUM bank of f32)

TRACE = False              # test harness sets True to collect an NTFF profile
LAST_RESULTS = None        # BassKernelResults of the last run (for test.py)

_NC_CACHE = {}


def _build_nc(normalize: bool):
    import concourse.bass as bass
    import concourse.mybir as mybir
    import concourse.tile as tile
    from concourse import bacc
    from concourse.masks import make_identity

    f32 = mybir.dt.float32
    f16 = mybir.dt.float16
    # Bacc (not plain Bass): its compile() runs the TRN2 legalization passes
    # (sync-wait splitting via event semaphores, matmul->ldweights wait moves,
    # reg alloc) that walrus codegen requires.
    nc = bacc.Bacc("TRN2", target_bir_lowering=False, debug=False,
                   enable_asserts=False)

    a_d = nc.declare_dram_parameter("a", [_AN, _D], f32, isOutput=False)
    b_d = nc.declare_dram_parameter("b", [_BM, _D], f32, isOutput=False)
    out_d = nc.declare_dram_parameter("out", [_AN, _BM], f32, isOutput=True)

    KA = _AN // _P             # 16 A tiles
    KB = _BM // _P             # 32 B tiles
    SQ = mybir.ActivationFunctionType.Square

    NCH = 4                    # B chunks (pipeline B-prep under the GEMM)
    CW = _BM // NCH            # output columns / B rows per chunk (1024)

    with tile.TileContext(nc) as tc:
        with (
            tc.tile_pool(name="const", bufs=1) as const_pool,
            tc.tile_pool(name="persist", bufs=1) as persist,
            tc.tile_pool(name="stage", bufs=3) as stage,
            tc.tile_pool(name="scal", bufs=4) as scal,
            tc.tile_pool(name="wpsum", bufs=1, space=bass.MemorySpace.PSUM) as wpsum,
            tc.tile_pool(name="tpsum", bufs=2, space=bass.MemorySpace.PSUM) as tpsum,
            tc.tile_pool(name="mpsum", bufs=5, space=bass.MemorySpace.PSUM) as mpsum,
            tc.tile_pool(name="ostage", bufs=3) as ostage,
        ):
            ident = const_pool.tile([_P, _P], f16)
            make_identity(nc, ident[:])

            # d-major (transposed) scaled fp16 operands, persistent in SBUF.
            aT = persist.tile([_P, _KC, _AN], f16)            # 2 MB
            bTc = [persist.tile([_P, _KC, CW], f16, name=f"bT{c}",
                                tag=f"bT{c}") for c in range(NCH)]   # 4 x 1 MB

            # Absorb the identity (GpSimd) dep before the transpose stream.
            warm = wpsum.tile([_P, _P], f32)
            nc.tensor.matmul(warm[:], lhsT=ident[:], rhs=ident[:],
                             start=True, stop=True)

            def prep(src, row0, n_tiles, dstT, col0):
                """Load nat tiles (rows row0..row0+128*n_tiles of src), scale
                rows by ssq^-1/4 (casting to fp16), transpose into dstT at
                column offset col0."""
                src_r = src[row0:row0 + n_tiles * _P, :].rearrange(
                    "(g j p) d -> g p j d", p=_P, j=4
                )
                for g in range(n_tiles // 4):
                    nat = stage.tile([_P, 4, _D], f32, tag="nat")
                    nc.sync.dma_start(nat[:], src_r[g])
                    for j in range(4):
                        t_idx = g * 4 + j
                        scaled = stage.tile([_P, _D], f16, tag="scaled")
                        if normalize:
                            sq = stage.tile([_P, _D], f32, tag="sq")
                            ssq = scal.tile([_P, 1], f32, tag="ssq")
                            nc.scalar.activation(sq[:], nat[:, j], SQ,
                                                 accum_out=ssq[:])
                            rec = scal.tile([_P, 1], f32, tag="rec")
                            nc.vector.reciprocal(rec[:], ssq[:])
                            s_half = scal.tile([_P, 1], f32, tag="s_half")
                            nc.scalar.sqrt(s_half[:], rec[:])
                            s_quart = scal.tile([_P, 1], f32, tag="s_quart")
                            nc.scalar.sqrt(s_quart[:], s_half[:])
                            nc.vector.tensor_scalar_mul(
                                scaled[:], in0=nat[:, j], scalar1=s_quart[:]
                            )
                        else:
                            nc.vector.tensor_copy(scaled[:], nat[:, j])
                        # Transpose [128 rows, 512 d] -> 4x [128 d, 128 rows]
                        # into one PSUM bank via matmuls against identity.
                        pt = tpsum.tile([_P, _KC, _P], f32)
                        for k in range(_KC):
                            nc.tensor.matmul(
                                pt[:, k],
                                lhsT=scaled[:, k * _P:(k + 1) * _P],
                                rhs=ident[:],
                                start=True,
                                stop=True,
                            )
                        co = col0 + t_idx * _P
                        nc.vector.tensor_copy(dstT[:, :, co:co + _P], pt[:])

            prep(a_d, 0, KA, aT, 0)

            # B-prep of chunk c+1 pipelines under the GEMM of chunk c.
            for c in range(NCH):
                prep(b_d, c * CW, CW // _P, bTc[c], 0)
                for t in range(KA):
                    ost = ostage.tile([_P, CW], f32)
                    for s in range(CW // _NS):
                        ps = mpsum.tile([_P, _NS], f32)
                        for k in range(_KC):
                            nc.tensor.matmul(
                                ps[:],
                                lhsT=aT[:, k, t * _P:(t + 1) * _P],
                                rhs=bTc[c][:, k, s * _NS:(s + 1) * _NS],
                                start=(k == 0),
                                stop=(k == _KC - 1),
                            )
                        # Split PSUM evacuation across DVE and ACT.
                        dst = ost[:, s * _NS:(s + 1) * _NS]
                        if (t * (CW // _NS) + s) % 3 == 2:
                            nc.scalar.copy(dst, ps[:])
                        else:
                            nc.vector.tensor_copy(dst, ps[:])
                    nc.sync.dma_start(
                        out_d[t * _P:(t + 1) * _P, c * CW:(c + 1) * CW], ost[:]
                    )

    nc.compile()
    return nc


def _get_nc(normalize: bool):
    key = bool(normalize)
    if key not in _NC_CACHE:
        _NC_CACHE[key] = _build_nc(key)
    return _NC_CACHE[key]


def kernel(first_vector, second_vector, normalize):
    global LAST_RESULTS
    from concourse.bass_utils import run_bass_kernel_spmd

    a = np.ascontiguousarray(np.asarray(first_vector, dtype=np.float32))
    b = np.ascontiguousarray(np.asarray(second_vector, dtype=np.float32))
    assert a.shape == (_N, _D) and b.shape == (_M, _D)
    norm = bool(int(np.asarray(normalize)))

    nc = _get_nc(norm)

    in_maps = []
    for c in range(_GRID_N * _GRID_M):
        ni, mj = divmod(c, _GRID_M)
        in_maps.append(
            {
                "a": a[ni * _AN:(ni + 1) * _AN],
                "b": b[mj * _BM:(mj + 1) * _BM],
            }
        )

    res = run_bass_kernel_spmd(
        nc, in_maps, core_ids=list(range(_GRID_N * _GRID_M)), trace=TRACE
    )
    LAST_RESULTS = res

    out = np.empty((_N, _M), dtype=np.float32)
    for c in range(_GRID_N * _GRID_M):
        ni, mj = divmod(c, _GRID_M)
        out[ni * _AN:(ni + 1) * _AN, mj * _BM:(mj + 1) * _BM] = res.results[c]["out"]
    return out


# revision 13
# speedup vs baseline: 3.1553x; 1.1002x over previous
"""Cosine-similarity (pairwise, normalized by sqrt(|a||b|)+eps) Trainium2 kernel.

Problem: first_vector [8192, 512] f32, second_vector [8192, 512] f32,
output sim [8192, 8192] f32 with
    sim = (A @ B.T) / (sqrt(|A_n| * |B_m|) + 1e-6)        (normalize=1)

Strategy (8 NeuronCores, SPMD, no collectives):
  * 2D shard: 4-way over A rows x 2-way over B rows. Core c=(ni,mj)
    computes the [2048, 4096] output slab at (ni*2048, mj*4096).
  * On-device: per-row sum-of-squares -> scale = ssq^(-1/4); pre-scale both
    A and B rows by their scale so the GEMM output is already normalized
    (the +eps in the reference denominator is dropped; rel. impact ~5e-8).
  * The scale multiply also casts to fp16: fp32 matmuls on TRN2 run at 1/4
    rate (2 weight passes x half-rate 4-byte rhs stream, ~860 ns per
    128x128x512 MM measured) while fp16 runs full rate with FWL weight
    loads. fp16 operand quantization costs ~3e-4 norm-relative error.
  * Transpose scaled fp16 tiles to d-major layout with PE matmuls against an
    fp16 identity (fp32 accumulate in PSUM is exact for fp16 values), then a
    dense fp16 GEMM: out[n, m] = sum_d aT[d, n] * bT[d, m] accumulated over
    4 k-chunks of 128 in PSUM (f32), evacuated to SBUF (alternating DVE /
    ACT) and streamed out with 512 KB contiguous DMAs.
  * The transposed operands are stored as fine-grained tiles (one per A
    row-tile, one per 512-wide B column group) so the GEMM's dependencies
    are per-tile: matmuls start as soon as the first transposes land
    instead of after the whole prep phase.
"""

import numpy as np

_N, _M, _D = 8192, 8192, 512
_P = 128
_GRID_N, _GRID_M = 4, 2
_AN = _N // _GRID_N        # A rows per core (2048)
_BM = _M // _GRID_M        # B rows per core (4096)
_KC = _D // _P             # contraction chunks (4)
_NS = 512                  # moving free dim per matmul (one PSUM bank of f32)

TRACE = False              # test harness sets True to collect an NTFF profile
LAST_RESULTS = None        # BassKernelResults of the last run (for test.py)

_NC_CACHE = {}


def _build_nc(normalize: bool):
    import concourse.bass as bass
    import concourse.mybir as mybir
    import concourse.tile as tile
    from concourse import bacc
    from concourse.masks import make_identity

    f32 = mybir.dt.float32
    f16 = mybir.dt.float16
    # Bacc (not plain Bass): its compile() runs the TRN2 legalization passes
    # (sync-wait splitting via event semaphores, matmul->ldweights wait moves,
    # reg alloc) that walrus codegen requires.
    nc = bacc.Bacc("TRN2", target_bir_lowering=False, debug=False,
                   enable_asserts=False)

    a_d = nc.declare_dram_parameter("a", [_AN, _D], f32, isOutput=False)
    b_d = nc.declare_dram_parameter("b", [_BM, _D], f32, isOutput=False)
    out_d = nc.declare_dram_parameter("out", [_AN, _BM], f32, isOutput=True)

    KA = _AN // _P             # 16 A row-tiles
    NSC = _BM // _NS           # 8 B column groups of 512
    SQ = mybir.ActivationFunctionType.Square

    with tile.TileContext(nc) as tc:
        with (
            tc.tile_pool(name="const", bufs=1) as const_pool,
            tc.tile_pool(name="persist", bufs=1) as persist,
            tc.tile_pool(name="stage", bufs=3) as stage,
            tc.tile_pool(name="scal", bufs=3) as scal,
            tc.tile_pool(name="wpsum", bufs=1, space=bass.MemorySpace.PSUM) as wpsum,
            tc.tile_pool(name="tpsum", bufs=2, space=bass.MemorySpace.PSUM) as tpsum,
            tc.tile_pool(name="mpsum", bufs=5, space=bass.MemorySpace.PSUM) as mpsum,
            tc.tile_pool(name="ostage", bufs=3) as ostage,
        ):
            ident = const_pool.tile([_P, _P], f16)
            make_identity(nc, ident[:])

            # Fine-grained d-major (transposed) scaled fp16 operands.
            aTt = [persist.tile([_P, _KC, _P], f16, name=f"aT{t}", tag=f"aT{t}")
                   for t in range(KA)]                       # 16 x 128 KB
            bTs = [persist.tile([_P, _KC, _NS], f16, name=f"bS{s}", tag=f"bS{s}")
                   for s in range(NSC)]                      # 8 x 512 KB

            # Absorb the identity (GpSimd) dep before the transpose stream.
            warm = wpsum.tile([_P, _P], f32)
            nc.tensor.matmul(warm[:], lhsT=ident[:], rhs=ident[:],
                             start=True, stop=True)

            def prep_group(src, row0, dst4):
                """Load 4 row-tiles (512 rows at row0), scale rows by
                ssq^-1/4 (casting to fp16), transpose each into dst4[j]
                (a (tile, column-offset) pair)."""
                src_r = src[row0:row0 + 4 * _P, :].rearrange(
                    "(j p) d -> p j d", p=_P
                )
                nat = stage.tile([_P, 4, _D], f32, tag="nat")
                nc.sync.dma_start(nat[:], src_r)
                if normalize:
                    ssq4 = scal.tile([_P, 4], f32, tag="ssq4")
                    for j in range(4):
                        sq = stage.tile([_P, _D], f32, tag="sq")
                        nc.scalar.activation(sq[:], nat[:, j], SQ,
                                             accum_out=ssq4[:, j:j + 1])
                    rec4 = scal.tile([_P, 4], f32, tag="rec4")
                    nc.vector.reciprocal(rec4[:], ssq4[:])
                    sh4 = scal.tile([_P, 4], f32, tag="sh4")
                    nc.scalar.sqrt(sh4[:], rec4[:])
                    s4 = scal.tile([_P, 4], f32, tag="s4")
                    nc.scalar.sqrt(s4[:], sh4[:])
                for j in range(4):
                    scaled = stage.tile([_P, _D], f16, tag="scaled")
                    if normalize:
                        nc.vector.tensor_scalar_mul(
                            scaled[:], in0=nat[:, j], scalar1=s4[:, j:j + 1]
                        )
                    else:
                        nc.vector.tensor_copy(scaled[:], nat[:, j])
                    pt = tpsum.tile([_P, _KC, _P], f32)
                    for k in range(_KC):
                        nc.tensor.matmul(
                            pt[:, k],
                            lhsT=scaled[:, k * _P:(k + 1) * _P],
                            rhs=ident[:],
                            start=True,
                            stop=True,
                        )
                    dstT, co = dst4[j]
                    nc.vector.tensor_copy(dstT[:, :, co:co + _P], pt[:])

            def prep_a(g):          # A row-tiles 4g..4g+3
                prep_group(a_d, g * 4 * _P,
                           [(aTt[g * 4 + j], 0) for j in range(4)])

            def prep_b(s):          # B column group s (rows 512s..512s+511)
                prep_group(b_d, s * _NS,
                           [(bTs[s], j * _P) for j in range(4)])

            # First the operands the GEMM touches first, then the rest —
            # fine-grained deps let matmuls start while later prep runs.
            prep_a(0)
            prep_b(0)
            prep_a(1)
            prep_b(1)
            prep_a(2)
            prep_a(3)
            for s in range(2, NSC):
                prep_b(s)

            # Main GEMM over column-group pairs: each ost is [128, 1024]
            # (512 KB store DMA). Pair p consumes bTs[2p], bTs[2p+1].
            cidx = 0
            for p in range(NSC // 2):
                for t in range(KA):
                    ost = ostage.tile([_P, 2 * _NS], f32)
                    for h in range(2):
                        s = 2 * p + h
                        ps = mpsum.tile([_P, _NS], f32)
                        for k in range(_KC):
                            nc.tensor.matmul(
                                ps[:],
                                lhsT=aTt[t][:, k, :],
                                rhs=bTs[s][:, k, :],
                                start=(k == 0),
                                stop=(k == _KC - 1),
                            )
                        # Split PSUM evacuation across DVE and ACT.
                        dst = ost[:, h * _NS:(h + 1) * _NS]
                        if cidx % 2 == 0:
                            nc.vector.tensor_copy(dst, ps[:])
                        else:
                            nc.scalar.copy(dst, ps[:])
                        cidx += 1
                    nc.sync.dma_start(
                        out_d[t * _P:(t + 1) * _P,
                              2 * p * _NS:(2 * p + 2) * _NS],
                        ost[:],
                    )

    nc.compile()
    return nc


def _get_nc(normalize: bool):
    key = bool(normalize)
    if key not in _NC_CACHE:
        _NC_CACHE[key] = _build_nc(key)
    return _NC_CACHE[key]


def kernel(first_vector, second_vector, normalize):
    global LAST_RESULTS
    from concourse.bass_utils import run_bass_kernel_spmd

    a = np.ascontiguousarray(np.asarray(first_vector, dtype=np.float32))
    b = np.ascontiguousarray(np.asarray(second_vector, dtype=np.float32))
    assert a.shape == (_N, _D) and b.shape == (_M, _D)
    norm = bool(int(np.asarray(normalize)))

    nc = _get_nc(norm)

    in_maps = []
    for c in range(_GRID_N * _GRID_M):
        ni, mj = divmod(c, _GRID_M)
        in_maps.append(
            {
                "a": a[ni * _AN:(ni + 1) * _AN],
                "b": b[mj * _BM:(mj + 1) * _BM],
            }
        )

    res = run_bass_kernel_spmd(
        nc, in_maps, core_ids=list(range(_GRID_N * _GRID_M)), trace=TRACE
    )
    LAST_RESULTS = res

    out = np.empty((_N, _M), dtype=np.float32)
    for c in range(_GRID_N * _GRID_M):
        ni, mj = divmod(c, _GRID_M)
        out[ni * _AN:(ni + 1) * _AN, mj * _BM:(mj + 1) * _BM] = res.results[c]["out"]
    return out


# revision 14
# speedup vs baseline: 3.2082x; 1.0167x over previous
"""Cosine-similarity (pairwise, normalized by sqrt(|a||b|)+eps) Trainium2 kernel.

Problem: first_vector [8192, 512] f32, second_vector [8192, 512] f32,
output sim [8192, 8192] f32 with
    sim = (A @ B.T) / (sqrt(|A_n| * |B_m|) + 1e-6)        (normalize=1)

Strategy (8 NeuronCores, SPMD, no collectives):
  * 2D shard: 4-way over A rows x 2-way over B rows. Core c=(ni,mj)
    computes the [2048, 4096] output slab at (ni*2048, mj*4096).
  * On-device: per-row sum-of-squares -> scale = ssq^(-1/4); pre-scale both
    A and B rows by their scale so the GEMM output is already normalized
    (the +eps in the reference denominator is dropped; rel. impact ~5e-8).
  * The scale multiply also casts to fp16: fp32 matmuls on TRN2 run at 1/4
    rate (2 weight passes x half-rate 4-byte rhs stream, ~860 ns per
    128x128x512 MM measured) while fp16 runs full rate with FWL weight
    loads. fp16 operand quantization costs ~3e-4 norm-relative error.
  * Transpose scaled fp16 tiles to d-major layout with PE matmuls against an
    fp16 identity (fp32 accumulate in PSUM is exact for fp16 values), then a
    dense fp16 GEMM: out[n, m] = sum_d aT[d, n] * bT[d, m] accumulated over
    4 k-chunks of 128 in PSUM (f32), evacuated to SBUF (alternating DVE /
    ACT) and streamed out with 512 KB contiguous DMAs.
  * The transposed operands are stored as fine-grained tiles (one per A
    row-tile, one per 512-wide B column group) so the GEMM's dependencies
    are per-tile: matmuls start as soon as the first transposes land
    instead of after the whole prep phase.
"""

import numpy as np

_N, _M, _D = 8192, 8192, 512
_P = 128
_GRID_N, _GRID_M = 4, 2
_AN = _N // _GRID_N        # A rows per core (2048)
_BM = _M // _GRID_M        # B rows per core (4096)
_KC = _D // _P             # contraction chunks (4)
_NS = 512                  # moving free dim per matmul (one PSUM bank of f32)

TRACE = False              # test harness sets True to collect an NTFF profile
LAST_RESULTS = None        # BassKernelResults of the last run (for test.py)

_NC_CACHE = {}


def _build_nc(normalize: bool):
    import concourse.bass as bass
    import concourse.mybir as mybir
    import concourse.tile as tile
    from concourse import bacc
    from concourse.masks import make_identity

    f32 = mybir.dt.float32
    f16 = mybir.dt.float16
    # Bacc (not plain Bass): its compile() runs the TRN2 legalization passes
    # (sync-wait splitting via event semaphores, matmul->ldweights wait moves,
    # reg alloc) that walrus codegen requires.
    nc = bacc.Bacc("TRN2", target_bir_lowering=False, debug=False,
                   enable_asserts=False)

    a_d = nc.declare_dram_parameter("a", [_AN, _D], f32, isOutput=False)
    b_d = nc.declare_dram_parameter("b", [_BM, _D], f32, isOutput=False)
    out_d = nc.declare_dram_parameter("out", [_AN, _BM], f32, isOutput=True)

    KA = _AN // _P             # 16 A row-tiles
    NSC = _BM // _NS           # 8 B column groups of 512
    SQ = mybir.ActivationFunctionType.Square

    with tile.TileContext(nc) as tc:
        with (
            tc.tile_pool(name="const", bufs=1) as const_pool,
            tc.tile_pool(name="persist", bufs=1) as persist,
            tc.tile_pool(name="stage", bufs=3) as stage,
            tc.tile_pool(name="scal", bufs=3) as scal,
            tc.tile_pool(name="wpsum", bufs=1, space=bass.MemorySpace.PSUM) as wpsum,
            tc.tile_pool(name="tpsum", bufs=2, space=bass.MemorySpace.PSUM) as tpsum,
            tc.tile_pool(name="mpsum", bufs=5, space=bass.MemorySpace.PSUM) as mpsum,
            tc.tile_pool(name="ostage", bufs=3) as ostage,
        ):
            ident = const_pool.tile([_P, _P], f16)
            make_identity(nc, ident[:])

            # Fine-grained d-major (transposed) scaled fp16 operands.
            aTt = [persist.tile([_P, _KC, _P], f16, name=f"aT{t}", tag=f"aT{t}")
                   for t in range(KA)]                       # 16 x 128 KB
            bTs = [persist.tile([_P, _KC, _NS], f16, name=f"bS{s}", tag=f"bS{s}")
                   for s in range(NSC)]                      # 8 x 512 KB

            # Absorb the identity (GpSimd) dep before the transpose stream.
            warm = wpsum.tile([_P, _P], f32)
            nc.tensor.matmul(warm[:], lhsT=ident[:], rhs=ident[:],
                             start=True, stop=True)

            def prep_group(src, row0, dst4):
                """Load 4 row-tiles (512 rows at row0), scale rows by
                ssq^-1/4 (casting to fp16), transpose each into dst4[j]
                (a (tile, column-offset) pair)."""
                src_r = src[row0:row0 + 4 * _P, :].rearrange(
                    "(j p) d -> p j d", p=_P
                )
                nat = stage.tile([_P, 4, _D], f32, tag="nat")
                nc.sync.dma_start(nat[:], src_r)
                if normalize:
                    ssq4 = scal.tile([_P, 4], f32, tag="ssq4")
                    for j in range(4):
                        sq = stage.tile([_P, _D], f32, tag="sq")
                        nc.scalar.activation(sq[:], nat[:, j], SQ,
                                             accum_out=ssq4[:, j:j + 1])
                    rec4 = scal.tile([_P, 4], f32, tag="rec4")
                    nc.vector.reciprocal(rec4[:], ssq4[:])
                    sh4 = scal.tile([_P, 4], f32, tag="sh4")
                    nc.scalar.sqrt(sh4[:], rec4[:])
                    s4 = scal.tile([_P, 4], f32, tag="s4")
                    nc.scalar.sqrt(s4[:], sh4[:])
                for j in range(4):
                    scaled = stage.tile([_P, _D], f16, tag="scaled")
                    if normalize:
                        nc.vector.tensor_scalar_mul(
                            scaled[:], in0=nat[:, j], scalar1=s4[:, j:j + 1]
                        )
                    else:
                        nc.vector.tensor_copy(scaled[:], nat[:, j])
                    pt = tpsum.tile([_P, _KC, _P], f32)
                    for k in range(_KC):
                        nc.tensor.matmul(
                            pt[:, k],
                            lhsT=scaled[:, k * _P:(k + 1) * _P],
                            rhs=ident[:],
                            start=True,
                            stop=True,
                        )
                    dstT, co = dst4[j]
                    nc.vector.tensor_copy(dstT[:, :, co:co + _P], pt[:])

            def prep_a(g):          # A row-tiles 4g..4g+3
                prep_group(a_d, g * 4 * _P,
                           [(aTt[g * 4 + j], 0) for j in range(4)])

            def prep_b(s):          # B column group s (rows 512s..512s+511)
                prep_group(b_d, s * _NS,
                           [(bTs[s], j * _P) for j in range(4)])

            # Prep exactly what the first two column pairs need up front;
            # the rest is emitted just-in-time inside the main loop so the
            # GEMM's PSUM-evacuation copies outrank it in scheduler priority
            # (prep emitted earlier would starve them and stall the PE on
            # full PSUM banks).
            prep_a(0)
            prep_b(0)
            prep_b(1)
            prep_a(1)
            prep_a(2)
            prep_a(3)
            prep_b(2)
            prep_b(3)

            # Main GEMM over column-group pairs: each ost is [128, 1024]
            # (512 KB store DMA). Pair p consumes bTs[2p], bTs[2p+1].
            cidx = 0
            for p in range(NSC // 2):
                for t in range(KA):
                    if t == 4 and 2 * p + 4 < NSC:
                        prep_b(2 * p + 4)        # pair p+2's operands,
                        prep_b(2 * p + 5)        # ~25us ahead of first use
                    ost = ostage.tile([_P, 2 * _NS], f32)
                    for h in range(2):
                        s = 2 * p + h
                        ps = mpsum.tile([_P, _NS], f32)
                        for k in range(_KC):
                            nc.tensor.matmul(
                                ps[:],
                                lhsT=aTt[t][:, k, :],
                                rhs=bTs[s][:, k, :],
                                start=(k == 0),
                                stop=(k == _KC - 1),
                            )
                        # Split PSUM evacuation across DVE and ACT.
                        dst = ost[:, h * _NS:(h + 1) * _NS]
                        if cidx % 2 == 0:
                            nc.vector.tensor_copy(dst, ps[:])
                        else:
                            nc.scalar.copy(dst, ps[:])
                        cidx += 1
                    nc.sync.dma_start(
                        out_d[t * _P:(t + 1) * _P,
                              2 * p * _NS:(2 * p + 2) * _NS],
                        ost[:],
                    )

    nc.compile()
    return nc


def _get_nc(normalize: bool):
    key = bool(normalize)
    if key not in _NC_CACHE:
        _NC_CACHE[key] = _build_nc(key)
    return _NC_CACHE[key]


def kernel(first_vector, second_vector, normalize):
    global LAST_RESULTS
    from concourse.bass_utils import run_bass_kernel_spmd

    a = np.ascontiguousarray(np.asarray(first_vector, dtype=np.float32))
    b = np.ascontiguousarray(np.asarray(second_vector, dtype=np.float32))
    assert a.shape == (_N, _D) and b.shape == (_M, _D)
    norm = bool(int(np.asarray(normalize)))

    nc = _get_nc(norm)

    in_maps = []
    for c in range(_GRID_N * _GRID_M):
        ni, mj = divmod(c, _GRID_M)
        in_maps.append(
            {
                "a": a[ni * _AN:(ni + 1) * _AN],
                "b": b[mj * _BM:(mj + 1) * _BM],
            }
        )

    res = run_bass_kernel_spmd(
        nc, in_maps, core_ids=list(range(_GRID_N * _GRID_M)), trace=TRACE
    )
    LAST_RESULTS = res

    out = np.empty((_N, _M), dtype=np.float32)
    for c in range(_GRID_N * _GRID_M):
        ni, mj = divmod(c, _GRID_M)
        out[ni * _AN:(ni + 1) * _AN, mj * _BM:(mj + 1) * _BM] = res.results[c]["out"]
    return out
